# revision 28
# baseline (speedup 1.0000x reference)
"""Trainium2 Bass kernel for CornerBoundingBoxEMDLoss.

For each sample: 8x8 pairwise corner distances, then exact min-cost perfect
matching via meet-in-the-middle (pred pairs -> quads -> complement join),
~50x less arithmetic than the reference's 8!-permutation GEMM.

v2 rewrite driven by the v1 NTFF trace (47.0us):
 - bf16 on the whole GEMM path (fp32 matmuls run double-pass LOW/HIGH on the
   PE: 2x cycles; one-hot weights are exact in bf16, dists only need ~1e-2).
 - pairwise distances via PE matmuls (transpose + signed-diff + square +
   k-sum selection GEMMs) instead of a GPSIMD broadcast add.
 - 3 input DMAs instead of 13 (DMA_DIRECT2D costs ~600ns each, serialized
   on SyncE): one packed constant blob, pred, targ.
 - output staged through a PE transpose so the final DMA is 4 contiguous
   512B descriptors instead of a 512 x 4B scatter (v1 lost ~8.7us there).
 - single activation table: square+sqrt+copy all live in act set 3
   ("sqrt_and_others"); a dummy sqrt first forces that table so we load
   tables once (v1 loaded twice, 1.3us each, one mid-pipeline).
 - PE warm-up matmuls during the DMA-wait head so real matmuls run at
   2.4GHz (HAM un-throttles after ~3.4us of sustained activity).
 - fused add+min70 join via tensor_tensor_reduce (one DVE op per chunk).

Data-parallel across 8 NeuronCores: 512 samples per core; samples on SBUF
partitions in 4 chunks of 128 for the quad/join stages, feature-major
(transposed) for the distance/pair stages.
"""

import itertools

import numpy as np

import concourse.bacc as bacc
import concourse.mybir as mybir
import concourse.tile as tile

N_CORES = 8
B_TOTAL = 4096
B_CORE = B_TOTAL // N_CORES          # 512
N_CHUNKS = 4
CHUNK = B_CORE // N_CHUNKS           # 128

F32 = mybir.dt.float32
BF16 = mybir.dt.bfloat16

N_WARMUP_MM = 0  # PE is power-capped at 1.2GHz here; warmups don't help
USE_DUMMY_SQRT = True
# bisect aid: 1 = stop after L1 (dump s1), 2 = full compute but v1-style
# scatter output (no transpose path), 3 = full kernel.
TRUNCATE = 3
# tensor_tensor_reduce with (add, min) hangs TRN2 hardware -- keep False.
USE_TTR = False

# ---- constant blobs (bf16). cs = small hot consts needed first (one
# fast DMA so phase 1 isn't gated on the big blob); cb = the rest.
CS_WA = 0              # [48, 96] diff selector, pred half A (i<4)
CS_WB = 96             # [48, 96] diff selector, pred half B (i>=4)
CS_KA = 192            # [96, 32] k-sum selector half A
CS_KB = 224            # [96, 32] k-sum selector half B
CS_COLS = 256
CB_IDENT = 0           # [128, 128] identity
CB_L2 = 128            # [112, 840] quad two-hot selectors
CB_L1O0 = 968          # [64, 112] pair selector, ordering 0
CB_L1O1 = 1080         # [64, 112] pair selector, ordering 1
CB_COLS = 1192


def _build_constants():
    pairs = list(itertools.combinations(range(8), 2))            # 28
    pair_idx = {p: i for i, p in enumerate(pairs)}
    subs4 = list(itertools.combinations(range(8), 4))            # 70
    pred_pairs = [(0, 1), (2, 3), (4, 5), (6, 7)]

    l1o0 = np.zeros((64, 112), dtype=np.float32)
    l1o1 = np.zeros((64, 112), dtype=np.float32)
    for q, (i0, i1) in enumerate(pred_pairs):
        for p, (a, b) in enumerate(pairs):
            col = q * 28 + p
            l1o0[i0 * 8 + a, col] = 1; l1o0[i1 * 8 + b, col] = 1
            l1o1[i0 * 8 + b, col] = 1; l1o1[i1 * 8 + a, col] = 1

    l2 = np.zeros((112, 840), dtype=np.float32)
    for t, T in enumerate(subs4):
        for s, S in enumerate(itertools.combinations(T, 2)):
            rest = tuple(sorted(set(T) - set(S)))
            l2[0 * 28 + pair_idx[S], t * 6 + s] = 1
            l2[1 * 28 + pair_idx[rest], t * 6 + s] = 1
        TB = tuple(sorted(set(range(8)) - set(T)))               # complement
        for s, S in enumerate(itertools.combinations(TB, 2)):
            rest = tuple(sorted(set(TB) - set(S)))
            l2[2 * 28 + pair_idx[S], 420 + t * 6 + s] = 1
            l2[3 * 28 + pair_idx[rest], 420 + t * 6 + s] = 1

    # diff selectors: ptt rows are pred coords (i*3+k) at partitions 0:24
    # and negated targ coords (24 + j*3+k) -- both transposed on the host.
    wa = np.zeros((48, 96), dtype=np.float32)
    wb = np.zeros((48, 96), dtype=np.float32)
    for i in range(4):
        for j in range(8):
            for k in range(3):
                wa[i * 3 + k, (i * 8 + j) * 3 + k] = 1
                wa[24 + j * 3 + k, (i * 8 + j) * 3 + k] = 1
                wb[(i + 4) * 3 + k, (i * 8 + j) * 3 + k] = 1
                wb[24 + j * 3 + k, (i * 8 + j) * 3 + k] = 1

    ka = np.zeros((96, 32), dtype=np.float32)
    kb = np.zeros((96, 32), dtype=np.float32)
    for p in range(32):
        for k in range(3):
            ka[p * 3 + k, p] = 1
            kb[p * 3 + k, p] = 1

    import ml_dtypes
    cs = np.zeros((96, CS_COLS), dtype=ml_dtypes.bfloat16)
    cs[:48, CS_WA:CS_WA + 96] = wa
    cs[:48, CS_WB:CS_WB + 96] = wb
    cs[:96, CS_KA:CS_KA + 32] = ka
    cs[:96, CS_KB:CS_KB + 32] = kb
    cb = np.zeros((128, CB_COLS), dtype=ml_dtypes.bfloat16)
    cb[:128, CB_IDENT:CB_IDENT + 128] = np.eye(128, dtype=np.float32)
    cb[:112, CB_L2:CB_L2 + 840] = l2
    cb[:64, CB_L1O0:CB_L1O0 + 112] = l1o0
    cb[:64, CB_L1O1:CB_L1O1 + 112] = l1o1
    return cs, cb


def build_nc():
    nc = bacc.Bacc("TRN2", target_bir_lowering=False, debug=False)
    AF = mybir.ActivationFunctionType

    cs_d = nc.dram_tensor("csmall", [96, CS_COLS], BF16, kind="ExternalInput")
    cb_d = nc.dram_tensor("cbig", [128, CB_COLS], BF16, kind="ExternalInput")
    # host-transposed inputs: rows 0:24 = predT coords, 24:48 = -targT
    ptt_d = nc.dram_tensor("ptt", [48, B_CORE], BF16, kind="ExternalInput")
    out_d = nc.dram_tensor("out", [B_CORE], F32, kind="ExternalOutput")

    with tile.TileContext(nc) as tc:
        with (
            tc.tile_pool(name="consts", bufs=1) as cpool,
            tc.tile_pool(name="persist", bufs=1) as ppool,
            tc.tile_pool(name="work", bufs=2) as wpool,
            # PSUM budget is 8 banks. psA: tpt(1) d2(1) ps0(1) ps1(1).
            # psB: two [128,1024] slots (2 banks each); the phase-1 diff
            # tiles and the per-chunk L2 tiles share them (diff is dead by
            # the time L2 starts), giving double-buffered L2 for free.
            tc.tile_pool(name="psA", bufs=1, space="PSUM") as psA,
            tc.tile_pool(name="psB", bufs=1, space="PSUM") as psB,
        ):
            cs = cpool.tile([96, CS_COLS], BF16, tag="cs")
            cb = cpool.tile([128, CB_COLS], BF16, tag="cb")
            ident = cb[:, CB_IDENT:CB_IDENT + 128]

            # ---- dummy 1-elem sqrt: hoists the sqrt act-table load to the
            # head of the ACT stream (overlaps the DMA wait).
            if USE_DUMMY_SQRT:
                dummy = cpool.tile([1, 2], F32, tag="dummy")
                nc.gpsimd.memset(dummy[:, :], 0.0)
                nc.scalar.activation(dummy[:, 0:1], dummy[:, 1:2], AF.Sqrt)

            nc.sync.dma_start(cs[:, :], cs_d[:, :])
            ptt = ppool.tile([48, B_CORE], BF16, tag="ptt")
            nc.scalar.dma_start(ptt[:, 0:256], ptt_d[:, 0:256])
            nc.sync.dma_start(ptt[:, 256:512], ptt_d[:, 256:512])
            nc.sync.dma_start(cb[:, :], cb_d[:, :])

            # ---- phase 1 + L1 in two sample-halves of 256 so the second
            # half's matmuls overlap the first half's ACT work ----
            slots = ["slotA", "slotB", "slotC"]
            snext = iter(slots[i % 3] for i in range(12)).__next__
            sq_a = ppool.tile([96, B_CORE], BF16, tag="sqa")
            sq_b = ppool.tile([96, B_CORE], BF16, tag="sqb")
            distT = ppool.tile([64, B_CORE], BF16, tag="distT")
            ps0 = psA.tile([112, B_CORE], F32, tag="ps0")
            ps1 = psA.tile([112, B_CORE], F32, tag="ps1")
            s1 = ppool.tile([112, B_CORE], F32, tag="s1")
            m_t = ppool.tile([112, B_CORE], BF16, tag="m")
            for h in range(2):
                hs = slice(h * 256, (h + 1) * 256)
                # signed diffs: diff[(ij),k] = pred[i,k] - targ[j,k]
                diff_a = psB.tile([96, 256], F32, tag=snext())
                diff_b = psB.tile([96, 256], F32, tag=snext())
                nc.tensor.matmul(diff_a[:, :], cs[0:48, CS_WA:CS_WA + 96],
                                 ptt[:, hs], start=True, stop=True)
                nc.tensor.matmul(diff_b[:, :], cs[0:48, CS_WB:CS_WB + 96],
                                 ptt[:, hs], start=True, stop=True)
                nc.scalar.activation(sq_a[:, hs], diff_a[:, :], AF.Square)
                nc.scalar.activation(sq_b[:, hs], diff_b[:, :], AF.Square)

                # k-sum -> squared distances (pair ij = i*8+j)
                d2a = psB.tile([32, 256], F32, tag=snext())
                d2b = psB.tile([32, 256], F32, tag=snext())
                nc.tensor.matmul(d2a[:, :], cs[0:96, CS_KA:CS_KA + 32],
                                 sq_a[:, hs], start=True, stop=True)
                nc.tensor.matmul(d2b[:, :], cs[0:96, CS_KB:CS_KB + 32],
                                 sq_b[:, hs], start=True, stop=True)
                nc.scalar.activation(distT[0:32, hs], d2a[:, :], AF.Sqrt)
                nc.scalar.activation(distT[32:64, hs], d2b[:, :], AF.Sqrt)

                # L1: pred-pair x target-pair costs, both orderings
                nc.tensor.matmul(ps0[:, hs], cb[0:64, CB_L1O0:CB_L1O0 + 112],
                                 distT[:, hs], start=True, stop=True)
                nc.tensor.matmul(ps1[:, hs], cb[0:64, CB_L1O1:CB_L1O1 + 112],
                                 distT[:, hs], start=True, stop=True)
                nc.scalar.activation(s1[:, hs], ps1[:, hs], AF.Copy)
                for c in (2 * h, 2 * h + 1):
                    sl = slice(c * CHUNK, (c + 1) * CHUNK)
                    nc.vector.tensor_tensor(m_t[:, sl], ps0[:, sl],
                                            s1[:, sl],
                                            op=mybir.AluOpType.min)

            if TRUNCATE == 1:
                nc.sync.dma_start(
                    out_d[:].rearrange("(c p) -> c p", c=4), s1[0:4, 0:128])

            # ---- L2 + join per chunk ----
            loss = ppool.tile([128, N_CHUNKS],
                              BF16 if TRUNCATE >= 3 else F32, tag="loss")
            for c in range(N_CHUNKS):
                sl = slice(c * CHUNK, (c + 1) * CHUNK)
                ps2 = psB.tile([128, 1024], F32, tag=snext())
                nc.tensor.matmul(ps2[:, 0:420], m_t[:, sl],
                                 cb[0:112, CB_L2:CB_L2 + 420],
                                 start=True, stop=True)
                nc.tensor.matmul(ps2[:, 512:932], m_t[:, sl],
                                 cb[0:112, CB_L2 + 420:CB_L2 + 840],
                                 start=True, stop=True)

                minab = wpool.tile([128, 140], BF16, tag="minab")
                v = (ps2[:, :].rearrange("p (h x) -> p h x", h=2)[:, :, 0:420]
                     .rearrange("p h (t s) -> p h t s", s=6))
                nc.vector.tensor_reduce(minab[:, :], v,
                                        axis=mybir.AxisListType.X,
                                        op=mybir.AluOpType.min)

                # fused: scratch = A + B(complement); loss = min over quads
                scratch = wpool.tile([128, 70], BF16, tag="scratch")
                if TRUNCATE >= 2:
                    if USE_TTR:
                        nc.vector.tensor_tensor_reduce(
                            scratch[:, :], minab[:, 0:70], minab[:, 70:140],
                            scale=1.0, scalar=1.0e30,
                            op0=mybir.AluOpType.add, op1=mybir.AluOpType.min,
                            accum_out=loss[:, c:c + 1])
                    else:
                        add_eng = nc.gpsimd if c < 2 else nc.vector
                        add_eng.tensor_tensor(
                            scratch[:, :], minab[:, 0:70], minab[:, 70:140],
                            op=mybir.AluOpType.add)
                        nc.vector.tensor_reduce(
                            loss[:, c:c + 1], scratch[:, :],
                            axis=mybir.AxisListType.X, op=mybir.AluOpType.min)

            if TRUNCATE == 2:
                # v1-style scatter output straight from loss (f32)
                nc.sync.dma_start(
                    out_d[:].rearrange("(c p) -> p c", p=128), loss[:, :])
            if TRUNCATE >= 3:
                # transpose so each DMA writes contiguous 512B rows; two
                # halves so the first DMA overlaps the second half's joins
                for h in range(2):
                    loss_ps = psA.tile([2, 128], BF16,
                                       tag=("ps0" if h == 0 else "ps1"))
                    nc.tensor.transpose(
                        loss_ps[:, :], loss[:, 2 * h:2 * h + 2], ident)
                    lossT = ppool.tile([2, 128], F32, tag=f"lossT{h}")
                    nc.scalar.activation(lossT[:, :], loss_ps[:, :], AF.Copy)
                    dma_eng = nc.scalar if h == 0 else nc.sync
                    dma_eng.dma_start(
                        out_d[256 * h:256 * (h + 1)]
                        .rearrange("(c p) -> c p", c=2), lossT[:, :])

    nc.compile()
    return nc


_NC = None


def _get_nc():
    global _NC
    if _NC is None:
        _NC = build_nc()
    return _NC


def _make_in_maps(pred_corners, target_corners):
    import ml_dtypes
    cs_blob, cb_blob = _build_constants()
    pred = np.ascontiguousarray(pred_corners, dtype=np.float32).reshape(B_TOTAL, 24)
    targn = -np.ascontiguousarray(target_corners, dtype=np.float32).reshape(B_TOTAL, 24)
    # transpose on the host: the kernel wants coords on partitions.
    ptt = np.concatenate([pred.T, targn.T], axis=0).astype(ml_dtypes.bfloat16)
    ptt = np.ascontiguousarray(ptt)  # [48, B_TOTAL]

    in_maps = []
    for k in range(N_CORES):
        sl = slice(k * B_CORE, (k + 1) * B_CORE)
        in_maps.append({"csmall": cs_blob, "cbig": cb_blob,
                        "ptt": np.ascontiguousarray(ptt[:, sl])})
    return in_maps


def kernel(pred_corners: np.ndarray, target_corners: np.ndarray) -> np.ndarray:
    from concourse.bass_utils import run_bass_kernel_spmd

    nc = _get_nc()
    in_maps = _make_in_maps(pred_corners, target_corners)
    res = run_bass_kernel_spmd(nc, in_maps, core_ids=list(range(N_CORES)))
    return np.concatenate([res.results[k]["out"] for k in range(N_CORES)])


# revision 29
# speedup vs baseline: 1.0408x; 1.0408x over previous
"""Trainium2 Bass kernel for CornerBoundingBoxEMDLoss.

For each sample: 8x8 pairwise corner distances, then exact min-cost perfect
matching via meet-in-the-middle (pred pairs -> quads -> complement join),
~50x less arithmetic than the reference's 8!-permutation GEMM.

v2 rewrite driven by the v1 NTFF trace (47.0us):
 - bf16 on the whole GEMM path (fp32 matmuls run double-pass LOW/HIGH on the
   PE: 2x cycles; one-hot weights are exact in bf16, dists only need ~1e-2).
 - pairwise distances via PE matmuls (transpose + signed-diff + square +
   k-sum selection GEMMs) instead of a GPSIMD broadcast add.
 - 3 input DMAs instead of 13 (DMA_DIRECT2D costs ~600ns each, serialized
   on SyncE): one packed constant blob, pred, targ.
 - output staged through a PE transpose so the final DMA is 4 contiguous
   512B descriptors instead of a 512 x 4B scatter (v1 lost ~8.7us there).
 - single activation table: square+sqrt+copy all live in act set 3
   ("sqrt_and_others"); a dummy sqrt first forces that table so we load
   tables once (v1 loaded twice, 1.3us each, one mid-pipeline).
 - PE warm-up matmuls during the DMA-wait head so real matmuls run at
   2.4GHz (HAM un-throttles after ~3.4us of sustained activity).
 - fused add+min70 join via tensor_tensor_reduce (one DVE op per chunk).

Data-parallel across 8 NeuronCores: 512 samples per core; samples on SBUF
partitions in 4 chunks of 128 for the quad/join stages, feature-major
(transposed) for the distance/pair stages.
"""

import itertools

import numpy as np

import concourse.bacc as bacc
import concourse.mybir as mybir
import concourse.tile as tile

N_CORES = 8
B_TOTAL = 4096
B_CORE = B_TOTAL // N_CORES          # 512
N_CHUNKS = 4
CHUNK = B_CORE // N_CHUNKS           # 128

F32 = mybir.dt.float32
BF16 = mybir.dt.bfloat16

N_WARMUP_MM = 0  # PE is power-capped at 1.2GHz here; warmups don't help
USE_DUMMY_SQRT = True
# bisect aid: 1 = stop after L1 (dump s1), 2 = full compute but v1-style
# scatter output (no transpose path), 3 = full kernel.
TRUNCATE = 3
# tensor_tensor_reduce with (add, min) hangs TRN2 hardware -- keep False.
USE_TTR = False

# ---- constant blobs (bf16). cs = small hot consts needed first (one
# fast DMA so phase 1 isn't gated on the big blob); cb = the rest.
CS_WA = 0              # [48, 96] diff selector, pred half A (i<4)
CS_WB = 96             # [48, 96] diff selector, pred half B (i>=4)
CS_KA = 192            # [96, 32] k-sum selector half A
CS_KB = 224            # [96, 32] k-sum selector half B
CS_COLS = 256
CB_IDENT = 0           # [128, 128] identity
CB_L2 = 128            # [112, 840] quad two-hot selectors
CB_L1O0 = 968          # [64, 112] pair selector, ordering 0
CB_L1O1 = 1080         # [64, 112] pair selector, ordering 1
CB_COLS = 1192


def _build_constants():
    pairs = list(itertools.combinations(range(8), 2))            # 28
    pair_idx = {p: i for i, p in enumerate(pairs)}
    subs4 = list(itertools.combinations(range(8), 4))            # 70
    pred_pairs = [(0, 1), (2, 3), (4, 5), (6, 7)]

    l1o0 = np.zeros((64, 112), dtype=np.float32)
    l1o1 = np.zeros((64, 112), dtype=np.float32)
    for q, (i0, i1) in enumerate(pred_pairs):
        for p, (a, b) in enumerate(pairs):
            col = q * 28 + p
            l1o0[i0 * 8 + a, col] = 1; l1o0[i1 * 8 + b, col] = 1
            l1o1[i0 * 8 + b, col] = 1; l1o1[i1 * 8 + a, col] = 1

    l2 = np.zeros((112, 840), dtype=np.float32)
    for t, T in enumerate(subs4):
        for s, S in enumerate(itertools.combinations(T, 2)):
            rest = tuple(sorted(set(T) - set(S)))
            l2[0 * 28 + pair_idx[S], t * 6 + s] = 1
            l2[1 * 28 + pair_idx[rest], t * 6 + s] = 1
        TB = tuple(sorted(set(range(8)) - set(T)))               # complement
        for s, S in enumerate(itertools.combinations(TB, 2)):
            rest = tuple(sorted(set(TB) - set(S)))
            l2[2 * 28 + pair_idx[S], 420 + t * 6 + s] = 1
            l2[3 * 28 + pair_idx[rest], 420 + t * 6 + s] = 1

    # diff selectors: ptt rows are pred coords (i*3+k) at partitions 0:24
    # and negated targ coords (24 + j*3+k) -- both transposed on the host.
    wa = np.zeros((48, 96), dtype=np.float32)
    wb = np.zeros((48, 96), dtype=np.float32)
    for i in range(4):
        for j in range(8):
            for k in range(3):
                wa[i * 3 + k, (i * 8 + j) * 3 + k] = 1
                wa[24 + j * 3 + k, (i * 8 + j) * 3 + k] = 1
                wb[(i + 4) * 3 + k, (i * 8 + j) * 3 + k] = 1
                wb[24 + j * 3 + k, (i * 8 + j) * 3 + k] = 1

    ka = np.zeros((96, 32), dtype=np.float32)
    kb = np.zeros((96, 32), dtype=np.float32)
    for p in range(32):
        for k in range(3):
            ka[p * 3 + k, p] = 1
            kb[p * 3 + k, p] = 1

    import ml_dtypes
    cs = np.zeros((96, CS_COLS), dtype=ml_dtypes.bfloat16)
    cs[:48, CS_WA:CS_WA + 96] = wa
    cs[:48, CS_WB:CS_WB + 96] = wb
    cs[:96, CS_KA:CS_KA + 32] = ka
    cs[:96, CS_KB:CS_KB + 32] = kb
    cb = np.zeros((128, CB_COLS), dtype=ml_dtypes.bfloat16)
    cb[:128, CB_IDENT:CB_IDENT + 128] = np.eye(128, dtype=np.float32)
    cb[:112, CB_L2:CB_L2 + 840] = l2
    cb[:64, CB_L1O0:CB_L1O0 + 112] = l1o0
    cb[:64, CB_L1O1:CB_L1O1 + 112] = l1o1
    return cs, cb


def build_nc():
    nc = bacc.Bacc("TRN2", target_bir_lowering=False, debug=False)
    AF = mybir.ActivationFunctionType

    cs_d = nc.dram_tensor("csmall", [96, CS_COLS], BF16, kind="ExternalInput")
    cb_d = nc.dram_tensor("cbig", [128, CB_COLS], BF16, kind="ExternalInput")
    # host-transposed inputs: rows 0:24 = predT coords, 24:48 = -targT
    ptt_d = nc.dram_tensor("ptt", [48, B_CORE], BF16, kind="ExternalInput")
    out_d = nc.dram_tensor("out", [B_CORE], F32, kind="ExternalOutput")

    with tile.TileContext(nc) as tc:
        with (
            tc.tile_pool(name="consts", bufs=1) as cpool,
            tc.tile_pool(name="persist", bufs=1) as ppool,
            tc.tile_pool(name="work", bufs=2) as wpool,
            # PSUM budget is 8 banks. psA: tpt(1) d2(1) ps0(1) ps1(1).
            # psB: two [128,1024] slots (2 banks each); the phase-1 diff
            # tiles and the per-chunk L2 tiles share them (diff is dead by
            # the time L2 starts), giving double-buffered L2 for free.
            tc.tile_pool(name="psA", bufs=1, space="PSUM") as psA,
            tc.tile_pool(name="psB", bufs=1, space="PSUM") as psB,
        ):
            cs = cpool.tile([96, CS_COLS], BF16, tag="cs")
            cb = cpool.tile([128, CB_COLS], BF16, tag="cb")
            ident = cb[:, CB_IDENT:CB_IDENT + 128]

            # ---- dummy 1-elem sqrt: hoists the sqrt act-table load to the
            # head of the ACT stream (overlaps the DMA wait).
            if USE_DUMMY_SQRT:
                dummy = cpool.tile([1, 2], F32, tag="dummy")
                nc.gpsimd.memset(dummy[:, :], 0.0)
                nc.scalar.activation(dummy[:, 0:1], dummy[:, 1:2], AF.Sqrt)

            nc.sync.dma_start(cs[:, :], cs_d[:, :])
            ptt = ppool.tile([48, B_CORE], BF16, tag="ptt")
            nc.scalar.dma_start(ptt[:, 0:256], ptt_d[:, 0:256])
            nc.sync.dma_start(ptt[:, 256:512], ptt_d[:, 256:512])
            nc.sync.dma_start(cb[:, :], cb_d[:, :])

            # ---- phase 1 + L1 in two sample-halves of 256 so the second
            # half's matmuls overlap the first half's ACT work ----
            slots = ["slotA", "slotB", "slotC"]
            snext = iter(slots[i % 3] for i in range(12)).__next__
            sq_a = ppool.tile([96, B_CORE], BF16, tag="sqa")
            sq_b = ppool.tile([96, B_CORE], BF16, tag="sqb")
            distT = ppool.tile([64, B_CORE], BF16, tag="distT")
            ps0 = psA.tile([112, B_CORE], F32, tag="ps0")
            ps1 = psA.tile([112, B_CORE], F32, tag="ps1")
            s1 = ppool.tile([112, B_CORE], F32, tag="s1")
            m_t = ppool.tile([112, B_CORE], BF16, tag="m")
            for h in range(2):
                hs = slice(h * 256, (h + 1) * 256)
                # signed diffs: diff[(ij),k] = pred[i,k] - targ[j,k]
                diff_a = psB.tile([96, 256], F32, tag=snext())
                diff_b = psB.tile([96, 256], F32, tag=snext())
                nc.tensor.matmul(diff_a[:, :], cs[0:48, CS_WA:CS_WA + 96],
                                 ptt[:, hs], start=True, stop=True)
                nc.tensor.matmul(diff_b[:, :], cs[0:48, CS_WB:CS_WB + 96],
                                 ptt[:, hs], start=True, stop=True)
                nc.scalar.activation(sq_a[:, hs], diff_a[:, :], AF.Square)
                nc.scalar.activation(sq_b[:, hs], diff_b[:, :], AF.Square)

                # k-sum -> squared distances (pair ij = i*8+j)
                d2a = psB.tile([32, 256], F32, tag=snext())
                d2b = psB.tile([32, 256], F32, tag=snext())
                nc.tensor.matmul(d2a[:, :], cs[0:96, CS_KA:CS_KA + 32],
                                 sq_a[:, hs], start=True, stop=True)
                nc.tensor.matmul(d2b[:, :], cs[0:96, CS_KB:CS_KB + 32],
                                 sq_b[:, hs], start=True, stop=True)
                nc.scalar.activation(distT[0:32, hs], d2a[:, :], AF.Sqrt)
                nc.scalar.activation(distT[32:64, hs], d2b[:, :], AF.Sqrt)

                # L1: pred-pair x target-pair costs, both orderings
                nc.tensor.matmul(ps0[:, hs], cb[0:64, CB_L1O0:CB_L1O0 + 112],
                                 distT[:, hs], start=True, stop=True)
                nc.tensor.matmul(ps1[:, hs], cb[0:64, CB_L1O1:CB_L1O1 + 112],
                                 distT[:, hs], start=True, stop=True)
                nc.scalar.activation(s1[:, hs], ps1[:, hs], AF.Copy)
                for c in (2 * h, 2 * h + 1):
                    sl = slice(c * CHUNK, (c + 1) * CHUNK)
                    nc.vector.tensor_tensor(m_t[:, sl], ps0[:, sl],
                                            s1[:, sl],
                                            op=mybir.AluOpType.min)

            if TRUNCATE == 1:
                nc.sync.dma_start(
                    out_d[:].rearrange("(c p) -> c p", c=4), s1[0:4, 0:128])

            # ---- L2 + join per chunk ----
            loss = ppool.tile([128, N_CHUNKS],
                              BF16 if TRUNCATE >= 3 else F32, tag="loss")
            for c in range(N_CHUNKS):
                sl = slice(c * CHUNK, (c + 1) * CHUNK)
                ps2 = psB.tile([128, 1024], F32, tag=snext())
                nc.tensor.matmul(ps2[:, 0:420], m_t[:, sl],
                                 cb[0:112, CB_L2:CB_L2 + 420],
                                 start=True, stop=True)
                nc.tensor.matmul(ps2[:, 512:932], m_t[:, sl],
                                 cb[0:112, CB_L2 + 420:CB_L2 + 840],
                                 start=True, stop=True)

                minab = wpool.tile([128, 140], BF16, tag="minab")
                v = (ps2[:, :].rearrange("p (h x) -> p h x", h=2)[:, :, 0:420]
                     .rearrange("p h (t s) -> p h t s", s=6))
                nc.vector.tensor_reduce(minab[:, :], v,
                                        axis=mybir.AxisListType.X,
                                        op=mybir.AluOpType.min)

                # fused: scratch = A + B(complement); loss = min over quads
                scratch = wpool.tile([128, 70], BF16, tag="scratch")
                if TRUNCATE >= 2:
                    if USE_TTR:
                        nc.vector.tensor_tensor_reduce(
                            scratch[:, :], minab[:, 0:70], minab[:, 70:140],
                            scale=1.0, scalar=1.0e30,
                            op0=mybir.AluOpType.add, op1=mybir.AluOpType.min,
                            accum_out=loss[:, c:c + 1])
                    else:
                        add_eng = nc.gpsimd if c < 3 else nc.vector
                        add_eng.tensor_tensor(
                            scratch[:, :], minab[:, 0:70], minab[:, 70:140],
                            op=mybir.AluOpType.add)
                        nc.vector.tensor_reduce(
                            loss[:, c:c + 1], scratch[:, :],
                            axis=mybir.AxisListType.X, op=mybir.AluOpType.min)

            if TRUNCATE == 2:
                # v1-style scatter output straight from loss (f32)
                nc.sync.dma_start(
                    out_d[:].rearrange("(c p) -> p c", p=128), loss[:, :])
            if TRUNCATE >= 3:
                # transpose so each DMA writes contiguous 512B rows; two
                # halves so the first DMA overlaps the second half's joins
                for h in range(2):
                    loss_ps = psA.tile([2, 128], BF16,
                                       tag=("ps0" if h == 0 else "ps1"))
                    nc.tensor.transpose(
                        loss_ps[:, :], loss[:, 2 * h:2 * h + 2], ident)
                    lossT = ppool.tile([2, 128], F32, tag=f"lossT{h}")
                    nc.scalar.activation(lossT[:, :], loss_ps[:, :], AF.Copy)
                    dma_eng = nc.scalar if h == 0 else nc.sync
                    dma_eng.dma_start(
                        out_d[256 * h:256 * (h + 1)]
                        .rearrange("(c p) -> c p", c=2), lossT[:, :])

    nc.compile()
    return nc


_NC = None


def _get_nc():
    global _NC
    if _NC is None:
        _NC = build_nc()
    return _NC


def _make_in_maps(pred_corners, target_corners):
    import ml_dtypes
    cs_blob, cb_blob = _build_constants()
    pred = np.ascontiguousarray(pred_corners, dtype=np.float32).reshape(B_TOTAL, 24)
    targn = -np.ascontiguousarray(target_corners, dtype=np.float32).reshape(B_TOTAL, 24)
    # transpose on the host: the kernel wants coords on partitions.
    ptt = np.concatenate([pred.T, targn.T], axis=0).astype(ml_dtypes.bfloat16)
    ptt = np.ascontiguousarray(ptt)  # [48, B_TOTAL]

    in_maps = []
    for k in range(N_CORES):
        sl = slice(k * B_CORE, (k + 1) * B_CORE)
        in_maps.append({"csmall": cs_blob, "cbig": cb_blob,
                        "ptt": np.ascontiguousarray(ptt[:, sl])})
    return in_maps


def kernel(pred_corners: np.ndarray, target_corners: np.ndarray) -> np.ndarray:
    from concourse.bass_utils import run_bass_kernel_spmd

    nc = _get_nc()
    in_maps = _make_in_maps(pred_corners, target_corners)
    res = run_bass_kernel_spmd(nc, in_maps, core_ids=list(range(N_CORES)))
    return np.concatenate([res.results[k]["out"] for k in range(N_CORES)])
